# revision 31
# baseline (speedup 1.0000x reference)
"""Trainium2 Bass kernel for the attention-LSTM caption decoder (8 cores).

Sharding: gate-dimension sharding of the recurrence (core k owns D-rows
[k*128,(k+1)*128) of each i/f/g/o gate block; weights SBUF-resident bf16;
full batch B=256 on the matmul moving dim). Attention is batch-sharded
(32 rows/core). aw/h1/h2 are exchanged via small intra-chip AllGathers
(bf16 payloads). The vocab projection is vocab-sharded and interleaved
into the recurrence so it fills AllGather latency windows.

Ragged truncation: lengths are sorted descending host-side, so active
rows form an exact prefix per step. Per-step active counts n_t are baked
into the instruction APs at compile time (the graph is fully unrolled):
gate GEMMs, pointwise ops, AG payloads and the vocab GEMM all shrink to
n_t columns. Inactive (t,b) outputs are zeroed host-side in kernel().

at_b is omitted deliberately: softmax(x + c) == softmax(x) exactly.
ad_b is folded into the img_att bias (af_b + ad_b) so it rides along.
"""

import numpy as np
import ml_dtypes

import concourse.bass as bass
import concourse.bacc as bacc
import concourse.tile as tile
from concourse import mybir
from concourse.bass_utils import run_bass_kernel_spmd

F32 = mybir.dt.float32
BF16 = mybir.dt.bfloat16
U8 = mybir.dt.uint8
AF = mybir.ActivationFunctionType

NC = 8
B, R, F = 256, 36, 2048
A, E, D, V = 512, 1024, 1024, 10000
L, T = 20, 19
BSH = B // NC            # 32
GS = 4 * (D // NC)       # 512 gate rows per core
VSH = 1280               # padded vocab shard
RB = R * BSH             # 1152
RG = [list(range(NC))]

_CACHED = {}


def bcast_r(ap, n):
    dims = list(ap.ap)
    dims.insert(1, [0, n])
    return bass.AP(tensor=ap.tensor, offset=ap.offset, ap=dims)


def mk_ap(ap, offset_elems, dims):
    return bass.AP(tensor=ap.tensor, offset=ap.offset + offset_elems, ap=dims)


def build(n_ts, nsteps=T, debug=False):
    n_ts = list(n_ts)
    assert len(n_ts) == nsteps
    nc = bacc.Bacc(None, target_bir_lowering=False, debug=False)

    def inp(name, shape, dt):
        return nc.dram_tensor(name, shape, dt, kind="ExternalInput")

    wtdh2 = inp("wtdh2", [D, GS], BF16)
    wtdh1 = inp("wtdh1", [D, GS], BF16)
    wfavg = inp("wfavg", [F, GS], BF16)
    we = inp("we", [E, GS], BF16)
    tdb = inp("tdb", [1, GS], F32)
    wlgaw = inp("wlgaw", [F, GS], BF16)
    wlgh1 = inp("wlgh1", [D, GS], BF16)
    wlgh2 = inp("wlgh2", [D, GS], BF16)
    lgb = inp("lgb", [1, GS], F32)
    adwT = inp("adwT", [D, A], BF16)
    bsel = inp("bsel", [B, BSH], BF16)
    eye32 = inp("eye32", [BSH, BSH], BF16)
    afw = inp("afw", [F, A], BF16)
    afb = inp("afb", [1, A], BF16)          # af_b + ad_b
    atw = inp("atw", [A, 1], BF16)
    ftsT = inp("ftsT", [F, RB], BF16)
    favgT = inp("favgT", [F, B], BF16)
    ftsrb = inp("ftsrb", [RB, F], BF16)
    embsT = inp("embsT", [E, T * B], BF16)
    outw = inp("outw", [D, VSH], BF16)
    outb = inp("outb", [VSH, 1], F32)
    bdiag = inp("bdiag", [128, BSH], BF16)
    t128 = inp("t128", [128, 128], BF16)

    outp = nc.dram_tensor("outp", [VSH, T * B], F32, kind="ExternalOutput")
    dbg = {}
    if debug:
        for nm, shp, dt_ in [("d_dec", [128, 4, BSH], BF16),
                             ("d_exps", [128, 9], F32),
                             ("d_arhs", [128, 9, BSH], BF16),
                             ("d_awb", [128, 16, BSH], BF16),
                             ("d_awg", [128, 16, B], BF16),
                             ("d_h1", [128, B], BF16),
                             ("d_h2", [128, B], BF16),
                             ("d_tdg", [128, 4, B], F32),
                             ("d_fav", [128, 4, B], F32),
                             ("d_lgg", [128, 4, B], F32)]:
            dbg[nm] = nc.dram_tensor(nm, shp, dt_, kind="ExternalOutput")

    def dram(name, shape, dt, shared=False):
        return nc.dram_tensor(name, shape, dt,
                              addr_space="Shared" if shared else "Local")

    agin_favg = dram("agin_favg", [F, BSH], F32)
    agout_favg = dram("agout_favg", [NC, F, BSH], F32, True)
    agin_aw = dram("agin_aw", [128, 16 * BSH], BF16)
    agout_aw = dram("agout_aw", [NC, 128, 16 * BSH], BF16, True)
    agin_h1 = dram("agin_h1", [128, B], BF16)
    agout_h1 = dram("agout_h1", [NC, 128, B], BF16, True)
    agin_h2 = dram("agin_h2", [128, B], BF16)
    agout_h2 = dram("agout_h2", [NC, 128, B], BF16, True)
    h2hist = dram("h2hist", [T, D, B], BF16)

    def ag(in_ap, out_ap):
        nc.gpsimd.collective_compute(
            "AllGather", mybir.AluOpType.bypass, replica_groups=RG,
            ins=[in_ap], outs=[out_ap])

    # vocab chunk groups: consecutive steps packed to <=256 active cols
    vgroups = []
    cur, cur_n = [], 0
    for t in range(nsteps):
        if cur_n + n_ts[t] > 256:
            vgroups.append(cur)
            cur, cur_n = [], 0
        cur.append(t)
        cur_n += n_ts[t]
    if cur:
        vgroups.append(cur)
    # group ready to emit at step (last step in group) + 2 so its h2hist
    # rows are already on-chip before the stall windows it should fill
    ready_at = {}
    for gi, g in enumerate(vgroups):
        ready_at.setdefault(g[-1] + 2, []).append(gi)

    with tile.TileContext(nc) as tc:
        with (
            tc.tile_pool(name="wp", bufs=1) as wp,
            tc.tile_pool(name="stp", bufs=1) as stp,
        ):
            def load_T(pool, src, kdim, name):
                t = pool.tile([128, kdim // 128, GS], BF16, tag=name)
                for kt in range(kdim // 128):
                    nc.sync.dma_start(
                        out=t[:, kt, :],
                        in_=mk_ap(src[:, :], kt * 128 * GS,
                                  [[GS, 128], [1, GS]]))
                return t

            wtdh2_s = load_T(wp, wtdh2, D, "wtdh2")
            wtdh1_s = load_T(wp, wtdh1, D, "wtdh1")
            we_s = load_T(wp, we, E, "we")
            wlgaw_s = load_T(wp, wlgaw, F, "wlgaw")
            wlgh1_s = load_T(wp, wlgh1, D, "wlgh1")
            wlgh2_s = load_T(wp, wlgh2, D, "wlgh2")
            adw_s = wp.tile([128, 8, A], BF16, tag="adw")
            for kt in range(8):
                nc.sync.dma_start(
                    out=adw_s[:, kt, :],
                    in_=mk_ap(adwT[:, :], kt * 128 * A,
                              [[A, 128], [1, A]]))
            outw_s = wp.tile([128, 8, VSH], BF16, tag="outw")
            for kt in range(8):
                nc.sync.dma_start(
                    out=outw_s[:, kt, :],
                    in_=mk_ap(outw[:, :], kt * 128 * VSH,
                              [[VSH, 128], [1, VSH]]))
            outb_s = wp.tile([128, 10], F32, tag="outb")
            nc.sync.dma_start(
                out=outb_s, in_=mk_ap(outb[:, :], 0, [[1, 128], [128, 10]]))
            tdb_s = wp.tile([128, 4], F32, tag="tdb")
            nc.sync.dma_start(out=tdb_s,
                              in_=mk_ap(tdb[:, :], 0, [[1, 128], [128, 4]]))
            lgb_s = wp.tile([128, 4], F32, tag="lgb")
            nc.sync.dma_start(out=lgb_s,
                              in_=mk_ap(lgb[:, :], 0, [[1, 128], [128, 4]]))
            bsel_s = wp.tile([128, 2, BSH], BF16, tag="bsel")
            nc.sync.dma_start(out=bsel_s,
                              in_=bsel[:, :].rearrange("(c p) j -> p c j", p=128))
            eye32_s = wp.tile([BSH, BSH], BF16, tag="eye32")
            nc.sync.dma_start(out=eye32_s, in_=eye32[:, :])
            atw_s = wp.tile([128, 4], BF16, tag="atw")
            nc.sync.dma_start(out=atw_s,
                              in_=mk_ap(atw[:, :], 0, [[1, 128], [128, 4]]))
            bdiag_s = wp.tile([128, BSH], BF16, tag="bdiag")
            nc.sync.dma_start(out=bdiag_s, in_=bdiag[:, :])
            t128_s = wp.tile([128, 128], BF16, tag="t128")
            nc.sync.dma_start(out=t128_s, in_=t128[:, :])
            ones_b = wp.tile([1, 384], BF16, tag="ones_b")
            nc.vector.memset(ones_b, 1.0)
            img_s = wp.tile([128, 4, RB], BF16, tag="img")
            favgp_s = wp.tile([128, 4, B], F32, tag="favgp")
            # fts SBUF-resident for the per-step attention einsum
            ftsr_s = wp.tile([128, 9, F], BF16, tag="ftsr")
            for c in range(9):
                nc.sync.dma_start(
                    out=ftsr_s[:, c, :],
                    in_=mk_ap(ftsrb[:, :], c * 128 * F, [[F, 128], [1, F]]))

            h1_s = stp.tile([128, B], F32, tag="h1")
            c1_s = stp.tile([128, B], F32, tag="c1")
            h2_s = stp.tile([128, B], F32, tag="h2")
            c2_s = stp.tile([128, B], F32, tag="c2")
            for s in (h1_s, c1_s, h2_s, c2_s):
                nc.vector.memset(s, 0.0)
            h1g_s = stp.tile([128, 8, B], BF16, tag="h1g")
            h2g_s = stp.tile([128, 8, B], BF16, tag="h2g")
            nc.vector.memset(h1g_s, 0.0)
            nc.vector.memset(h2g_s, 0.0)
            h1b_s = stp.tile([128, B], BF16, tag="h1b")
            h2b_s = stp.tile([128, B], BF16, tag="h2b")

            # ================= phase 0 =================
            with (
                tc.tile_pool(name="p0", bufs=2) as p0,
                tc.tile_pool(name="p0w", bufs=1) as p0w,
                tc.tile_pool(name="p0ps", bufs=1, space="PSUM") as p0ps,
            ):
                afb_t = p0.tile([1, A], BF16, tag="afb")
                nc.sync.dma_start(out=afb_t, in_=afb[:, :])
                # img_att = af_w @ fts^T + (af_b + ad_b); streamed over kt
                for nch in range(3):
                    ia_ps = p0ps.tile([128, 4, 512], F32, tag="iaps")
                    for kt in range(16):
                        afw_c = p0.tile([128, A], BF16, tag="afwc")
                        nc.sync.dma_start(
                            out=afw_c,
                            in_=mk_ap(afw[:, :], kt * 128 * A,
                                      [[A, 128], [1, A]]))
                        fts_c = p0.tile([128, 384], BF16, tag="ftsc")
                        nc.sync.dma_start(
                            out=fts_c,
                            in_=mk_ap(ftsT[:, :], kt * 128 * RB + nch * 384,
                                      [[RB, 128], [1, 384]]))
                        for m in range(4):
                            nc.tensor.matmul(
                                ia_ps[:, m, 0:384],
                                afw_c[:, m * 128:(m + 1) * 128],
                                fts_c[:, :], start=(kt == 0), stop=False)
                    for m in range(4):
                        nc.tensor.matmul(
                            ia_ps[:, m, 0:384], afb_t[:, m * 128:(m + 1) * 128],
                            ones_b[:, :], start=False, stop=True)
                        nc.scalar.copy(
                            img_s[:, m, nch * 384:(nch + 1) * 384],
                            ia_ps[:, m, 0:384])
                # favg gate contribution from host-computed favgT
                favgg = p0w.tile([128, 16, B], BF16, tag="favgg")
                nc.sync.dma_start(
                    out=favgg,
                    in_=mk_ap(favgT[:, :], 0,
                              [[B, 128], [128 * B, 16], [1, B]]))
                fp_ps = p0ps.tile([128, 4, 512], F32, tag="fpps")
                for kt in range(16):
                    wfavg_c = p0.tile([128, GS], BF16, tag="wfavgc")
                    nc.sync.dma_start(
                        out=wfavg_c,
                        in_=mk_ap(wfavg[:, :], kt * 128 * GS,
                                  [[GS, 128], [1, GS]]))
                    for m in range(4):
                        nc.tensor.matmul(
                            fp_ps[:, m, 0:B],
                            wfavg_c[:, m * 128:(m + 1) * 128],
                            favgg[:, kt, :], start=(kt == 0), stop=(kt == 15))
                for m in range(4):
                    nc.vector.tensor_scalar_add(favgp_s[:, m, :],
                                                fp_ps[:, m, 0:B],
                                                tdb_s[:, m:m + 1])

            # ================= phase 1: recurrence =================
            with (
                tc.tile_pool(name="p1", bufs=2) as p1,
                tc.tile_pool(name="pawg", bufs=1) as pawg,
                tc.tile_pool(name="pemb", bufs=2) as pemb,
                tc.tile_pool(name="pst", bufs=2) as pst,
                tc.tile_pool(name="pse2", bufs=1, space="PSUM") as pse2,
                tc.tile_pool(name="p1c", bufs=1) as p1c,
                tc.tile_pool(name="pv", bufs=2) as pv,
                tc.tile_pool(name="psg", bufs=1, space="PSUM") as psg,
                tc.tile_pool(name="psa", bufs=2, space="PSUM") as psa,
            ):
                vload = {}

                def load_vchunk(g):
                    ts_list = vgroups[g]
                    segs = []
                    off = 0
                    for tt in ts_list:
                        segs.append((tt, off, n_ts[tt]))
                        off += n_ts[tt]
                    N = off
                    h2v = pv.tile([128, 8, 256], BF16, tag="h2v")
                    for (tt, o, n) in segs:
                        nc.scalar.dma_start(
                            out=h2v[:, :, o:o + n],
                            in_=mk_ap(h2hist[:, :, :], tt * D * B,
                                      [[B, 128], [128 * B, 8], [1, n]]))
                    vload[g] = (segs, N, h2v)

                def emit_vchunk(g, m0, m1):
                    segs, N, h2v = vload[g]
                    for m in range(m0, m1):
                        vps = psa.tile([128, 512], F32, tag="small")
                        for kt in range(8):
                            nc.tensor.matmul(
                                vps[:, 0:N],
                                outw_s[:, kt, m * 128:(m + 1) * 128],
                                h2v[:, kt, 0:N],
                                start=(kt == 0), stop=(kt == 7))
                        ls = pv.tile([128, 256], F32, tag="ls")
                        nc.vector.tensor_scalar_add(ls[:, 0:N], vps[:, 0:N],
                                                    outb_s[:, m:m + 1])
                        for (tt, o, n) in segs:
                            nc.scalar.dma_start(
                                out=mk_ap(outp[:, :],
                                          m * 128 * T * B + tt * B,
                                          [[T * B, 128], [1, n]]),
                                in_=ls[:, o:o + n])

                embp_stash = {}

                def emit_embpre(tn):
                    # we @ emb(tn) (+ favgp) stashed to SBUF during step
                    # tn-1's aw-AG stall window
                    nt1 = n_ts[tn]
                    emb_t = pemb.tile([128, 8, B], BF16, tag="embt")
                    nc.sync.dma_start(
                        out=emb_t[:, :, 0:nt1],
                        in_=mk_ap(embsT[:, :], tn * B,
                                  [[T * B, 128], [128 * T * B, 8], [1, nt1]]))
                    eps = pse2.tile([128, 4, 256], F32, tag="eps")
                    for m in range(4):
                        sl = slice(m * 128, (m + 1) * 128)
                        for kt in range(8):
                            nc.tensor.matmul(eps[:, m, 0:nt1],
                                             we_s[:, kt, sl],
                                             emb_t[:, kt, 0:nt1],
                                             start=(m in (0, 2) and kt == 0),
                                             stop=(m in (1, 3) and kt == 7),
                                             skip_group_check=True)
                    stash = pst.tile([128, 4, B], F32, tag="embp")
                    nc.vector.tensor_add(stash[:, :, 0:nt1],
                                         eps[:, :, 0:nt1],
                                         favgp_s[:, :, 0:nt1])
                    embp_stash[tn] = stash

                for t in range(nsteps):
                    nt = n_ts[t]
                    nbc = (nt + 127) // 128      # 128-col chunks for dec
                    nsl = (nt + BSH - 1) // BSH  # 32-col slabs for aw gather
                    for g in ready_at.get(t, []):
                        load_vchunk(g)
                        emit_vchunk(g, 0, 5)

                    if t == 0:
                        emb_t = pemb.tile([128, 8, B], BF16, tag="embt")
                        nc.sync.dma_start(
                            out=emb_t[:, :, 0:nt],
                            in_=mk_ap(embsT[:, :], t * B,
                                      [[T * B, 128], [128 * T * B, 8],
                                       [1, nt]]))

                    # ---- td-LSTM gates (h1+emb first, h2 last) ----
                    tdps = psg.tile([128, 4, 512], F32, tag="gps")
                    for m in range(4):
                        sl = slice(m * 128, (m + 1) * 128)
                        for kt in range(8):
                            nc.tensor.matmul(tdps[:, m, 0:nt],
                                             wtdh1_s[:, kt, sl],
                                             h1g_s[:, kt, 0:nt],
                                             start=(kt == 0), stop=False)
                        if t == 0:
                            for kt in range(8):
                                nc.tensor.matmul(tdps[:, m, 0:nt],
                                                 we_s[:, kt, sl],
                                                 emb_t[:, kt, 0:nt],
                                                 start=False, stop=False)
                    for m in range(4):
                        sl = slice(m * 128, (m + 1) * 128)
                        for kt in range(8):
                            nc.tensor.matmul(tdps[:, m, 0:nt],
                                             wtdh2_s[:, kt, sl],
                                             h2g_s[:, kt, 0:nt],
                                             start=False, stop=(kt == 7))
                    tdv = tdps[:, :, 0:nt]
                    if t == 0:
                        nc.vector.tensor_add(tdv, tdv, favgp_s[:, :, 0:nt])
                    else:
                        nc.vector.tensor_add(tdv, tdv,
                                             embp_stash[t][:, :, 0:nt])
                    if debug and t == 0:
                        dtdg = p1c.tile([128, 4, B], F32, tag="dtdg")
                        nc.vector.tensor_copy(dtdg[:, :, 0:nt], tdv)
                        nc.sync.dma_start(out=dbg["d_tdg"].ap(), in_=dtdg)
                        nc.sync.dma_start(out=dbg["d_fav"].ap(), in_=favgp_s)
                    gi = p1c.tile([128, B], F32, tag="g0")
                    nc.scalar.activation(gi[:, 0:nt], tdps[:, 0, 0:nt], AF.Sigmoid)
                    gf = p1c.tile([128, B], F32, tag="g1")
                    nc.scalar.activation(gf[:, 0:nt], tdps[:, 1, 0:nt], AF.Sigmoid)
                    gg = p1c.tile([128, B], F32, tag="g2")
                    nc.scalar.activation(gg[:, 0:nt], tdps[:, 2, 0:nt], AF.Tanh)
                    go = p1c.tile([128, B], F32, tag="g3")
                    nc.scalar.activation(go[:, 0:nt], tdps[:, 3, 0:nt], AF.Sigmoid)
                    t1 = p1c.tile([128, B], F32, tag="x")
                    nc.vector.tensor_mul(t1[:, 0:nt], gf[:, 0:nt], c1_s[:, 0:nt])
                    t2 = p1c.tile([128, B], F32, tag="y")
                    nc.vector.tensor_mul(t2[:, 0:nt], gi[:, 0:nt], gg[:, 0:nt])
                    nc.vector.tensor_add(c1_s[:, 0:nt], t1[:, 0:nt], t2[:, 0:nt])
                    tc1 = p1c.tile([128, B], F32, tag="g1")
                    nc.scalar.activation(tc1[:, 0:nt], c1_s[:, 0:nt], AF.Tanh)
                    nc.vector.tensor_mul(h1_s[:, 0:nt], go[:, 0:nt], tc1[:, 0:nt])

                    # ---- h1 allgather (lands during attention) ----
                    nc.vector.tensor_copy(h1b_s[:, 0:nt], h1_s[:, 0:nt])
                    nc.sync.dma_start(
                        out=mk_ap(agin_h1[:, :], 0, [[nt, 128], [1, nt]]),
                        in_=h1b_s[:, 0:nt])
                    ag(mk_ap(agin_h1[:, :], 0, [[nt, 128], [1, nt]]),
                       mk_ap(agout_h1[:, :, :], 0,
                             [[128 * nt, NC], [nt, 128], [1, nt]]))
                    nc.sync.dma_start(
                        out=h1g_s[:, :, 0:nt],
                        in_=mk_ap(agout_h1[:, :, :], 0,
                                  [[nt, 128], [128 * nt, 8], [1, nt]]))

                    # ---- dec_att via transposed GEMM + input-driven select
                    dtp = psg.tile([128, 2, 512], F32, tag="gps")
                    for kt in range(8):
                        for bc in range(nbc):
                            nc.tensor.matmul(
                                dtp[:, bc, :],
                                h1g_s[:, kt, bc * 128:bc * 128 + 128],
                                adw_s[:, kt, :], start=(kt == 0), stop=(kt == 7))
                    decT_sb = p1c.tile([128, 2, A], BF16, tag="decT")
                    nc.scalar.copy(decT_sb[:, 0:nbc, :], dtp[:, 0:nbc, :])
                    dsp = psa.tile([BSH, A], F32, tag="small")
                    for bc in range(nbc):
                        nc.tensor.matmul(dsp[:, :], bsel_s[:, bc, :],
                                         decT_sb[:, bc, :],
                                         start=(bc == 0), stop=(bc == nbc - 1))
                    dsel_sb = p1c.tile([BSH, A], BF16, tag="dsel")
                    nc.scalar.copy(dsel_sb, dsp)
                    ttp = psa.tile([128, 4, BSH], BF16, tag="small")
                    for q in range(4):
                        nc.tensor.transpose(
                            ttp[:, q, :],
                            dsel_sb[:, q * 128:(q + 1) * 128],
                            eye32_s[:, :])
                    dec = p1c.tile([128, 4, BSH], BF16, tag="dec")
                    nc.scalar.copy(dec, ttp)

                    # ---- attention scores + softmax ----
                    scps = psa.tile([128, 40], F32, tag="small")
                    rel4 = p1c.tile([128, 4, RB], BF16, tag="rel4")
                    dec_ap = dec[:, :, :]
                    dec_b = bass.AP(
                        tensor=dec_ap.tensor, offset=dec_ap.offset,
                        ap=[dec_ap.ap[0], [BSH, 4], [0, R], [1, BSH]])
                    nc.vector.tensor_add(
                        rel4.rearrange("p m (r b) -> p m r b", r=R),
                        img_s[:, :, :].rearrange("p m (r b) -> p m r b", r=R),
                        dec_b)
                    nc.vector.tensor_scalar_max(rel4, rel4, 0.0)
                    for m in range(4):
                        rel = rel4[:, m, :]
                        for c in range(9):
                            nc.tensor.matmul(
                                scps[:, m * 9 + c:m * 9 + c + 1],
                                rel[:, c * 128:(c + 1) * 128],
                                atw_s[:, m:m + 1],
                                start=True, stop=True)
                    scs = p1c.tile([128, 9], F32, tag="scs")
                    nc.vector.reduce_sum(
                        out=scs,
                        in_=scps[:, 0:36].rearrange("p (m c) -> p c m", m=4),
                        axis=mybir.AxisListType.X)
                    exps = p1c.tile([128, 9], F32, tag="exps")
                    nc.scalar.activation(exps, scs, AF.Exp)
                    expb = p1c.tile([128, 9], BF16, tag="expb")
                    nc.vector.tensor_copy(expb, exps)
                    for c in range(9):
                        nc.tensor.matmul(scps[:, 36:37], t128_s[:, :],
                                         expb[:, c:c + 1],
                                         start=(c == 0), stop=(c == 8))
                    rinv = p1c.tile([128, 1], F32, tag="rinv")
                    nc.vector.reciprocal(rinv, scps[:, 36:37])
                    arhs = p1c.tile([128, 9, BSH], BF16, tag="arhs")
                    for c in range(9):
                        nc.vector.tensor_scalar(
                            arhs[:, c, :], bdiag_s, exps[:, c:c + 1],
                            rinv[:, 0:1], mybir.AluOpType.mult,
                            mybir.AluOpType.mult)
                    # ---- aw einsum, transposed: out awT [32b, 2048f] ----
                    awtp = psg.tile([BSH, 16, 128], F32, tag="gps")
                    for c in range(9):
                        for q in range(4):
                            nc.tensor.matmul(
                                awtp[:, q * 4:(q + 1) * 4, :],
                                arhs[:, c, :],
                                ftsr_s[:, c, q * 512:(q + 1) * 512],
                                start=(c == 0), stop=(c == 8))
                    awtb = p1c.tile([BSH, 16, 128], BF16, tag="awtb")
                    nc.scalar.copy(awtb, awtp)
                    # transpose back to [128f, 16, 32b]
                    awtt = psa.tile([128, 8, BSH], BF16, tag="small")
                    awb = p1c.tile([128, 16, BSH], BF16, tag="awb")
                    for h in range(2):
                        for fc in range(8):
                            nc.tensor.transpose(
                                awtt[:, fc, :],
                                awtb[:, h * 8 + fc, :],
                                eye32_s[:, :])
                        nc.scalar.copy(awb[:, h * 8:(h + 1) * 8, :], awtt)
                    # agin_aw DRAM layout: [128p, kt*32+c] contiguous per
                    # partition (1 KiB runs) on both AG sides.
                    nc.sync.dma_start(
                        out=mk_ap(agin_aw[:, :], 0,
                                  [[16 * BSH, 128], [1, 16 * BSH]]),
                        in_=awb)
                    ag(mk_ap(agin_aw[:, :], 0,
                             [[BSH * 16, 128], [1, BSH * 16]]),
                       mk_ap(agout_aw[:, :, :], 0,
                             [[128 * BSH * 16, NC], [BSH * 16, 128],
                              [1, BSH * 16]]))

                    # ---- lg-LSTM: h1+h2 terms fill the aw-AG window ----
                    lgps = psg.tile([128, 4, 512], F32, tag="gps")
                    for m in range(4):
                        sl = slice(m * 128, (m + 1) * 128)
                        for kt in range(8):
                            nc.tensor.matmul(lgps[:, m, 0:nt],
                                             wlgh1_s[:, kt, sl],
                                             h1g_s[:, kt, 0:nt],
                                             start=(kt == 0), stop=False)
                        for kt in range(8):
                            nc.tensor.matmul(lgps[:, m, 0:nt],
                                             wlgh2_s[:, kt, sl],
                                             h2g_s[:, kt, 0:nt],
                                             start=False, stop=False)
                    for g in ready_at.get(t, []):
                        emit_vchunk(g, 5, 10)
                    if t + 1 < nsteps:
                        emit_embpre(t + 1)
                    # gather-back: rank-blocked [128p, r, kt*32+c], one DMA,
                    # 1 KiB contiguous runs
                    awg = pawg.tile([128, 8, 16 * BSH], BF16, tag="awg")
                    nc.sync.dma_start(
                        out=awg[:, 0:nsl, :],
                        in_=mk_ap(agout_aw[:, :, :], 0,
                                  [[16 * BSH, 128], [128 * 16 * BSH, nsl],
                                   [1, 16 * BSH]]))
                    ntp = nsl * BSH
                    awg_ap = awg[:, :, :]
                    for m in range(4):
                        sl = slice(m * 128, (m + 1) * 128)
                        for kt in range(16):
                            rhs = bass.AP(
                                tensor=awg_ap.tensor,
                                offset=awg_ap.offset + kt * BSH,
                                ap=[awg_ap.ap[0], [16 * BSH, nsl], [1, BSH]])
                            nc.tensor.matmul(lgps[:, m, 0:ntp],
                                             wlgaw_s[:, kt, sl],
                                             rhs,
                                             start=False, stop=(kt == 15))
                    if debug and t == 0:
                        dlgg = p1c.tile([128, 4, B], F32, tag="dtdg")
                        nc.vector.tensor_copy(dlgg[:, :, 0:nt], lgps[:, :, 0:nt])
                        nc.sync.dma_start(out=dbg["d_lgg"].ap(), in_=dlgg)
                    gi2 = p1c.tile([128, B], F32, tag="g0")
                    nc.scalar.activation(gi2[:, 0:nt], lgps[:, 0, 0:nt],
                                         AF.Sigmoid, bias=lgb_s[:, 0:1])
                    gf2 = p1c.tile([128, B], F32, tag="g1")
                    nc.scalar.activation(gf2[:, 0:nt], lgps[:, 1, 0:nt],
                                         AF.Sigmoid, bias=lgb_s[:, 1:2])
                    gg2 = p1c.tile([128, B], F32, tag="g2")
                    nc.scalar.activation(gg2[:, 0:nt], lgps[:, 2, 0:nt],
                                         AF.Tanh, bias=lgb_s[:, 2:3])
                    go2 = p1c.tile([128, B], F32, tag="g3")
                    nc.scalar.activation(go2[:, 0:nt], lgps[:, 3, 0:nt],
                                         AF.Sigmoid, bias=lgb_s[:, 3:4])
                    t3 = p1c.tile([128, B], F32, tag="x")
                    nc.vector.tensor_mul(t3[:, 0:nt], gf2[:, 0:nt], c2_s[:, 0:nt])
                    t4 = p1c.tile([128, B], F32, tag="y")
                    nc.vector.tensor_mul(t4[:, 0:nt], gi2[:, 0:nt], gg2[:, 0:nt])
                    nc.vector.tensor_add(c2_s[:, 0:nt], t3[:, 0:nt], t4[:, 0:nt])
                    tc2 = p1c.tile([128, B], F32, tag="g1")
                    nc.scalar.activation(tc2[:, 0:nt], c2_s[:, 0:nt], AF.Tanh)
                    nc.vector.tensor_mul(h2_s[:, 0:nt], go2[:, 0:nt], tc2[:, 0:nt])

                    if debug and t == 0:
                        for nm, tl in [("d_dec", dec), ("d_exps", exps),
                                       ("d_arhs", arhs), ("d_awb", awb),
                                       ("d_awg", awg), ("d_h1", h1b_s)]:
                            nc.sync.dma_start(out=dbg[nm].ap(), in_=tl)
                    # ---- h2 allgather + history ----
                    nc.vector.tensor_copy(h2b_s[:, 0:nt], h2_s[:, 0:nt])
                    nc.sync.dma_start(
                        out=mk_ap(agin_h2[:, :], 0, [[nt, 128], [1, nt]]),
                        in_=h2b_s[:, 0:nt])
                    ag(mk_ap(agin_h2[:, :], 0, [[nt, 128], [1, nt]]),
                       mk_ap(agout_h2[:, :, :], 0,
                             [[128 * nt, NC], [nt, 128], [1, nt]]))
                    nc.sync.dma_start(
                        out=h2g_s[:, :, 0:nt],
                        in_=mk_ap(agout_h2[:, :, :], 0,
                                  [[nt, 128], [128 * nt, 8], [1, nt]]))
                    nc.sync.dma_start(
                        out=mk_ap(h2hist[:, :, :], t * D * B,
                                  [[B, D], [1, nt]]),
                        in_=mk_ap(agout_h2[:, :, :], 0,
                                  [[nt, D], [1, nt]]))
                    if debug and t == 0:
                        nc.sync.dma_start(out=dbg["d_h2"].ap(), in_=h2b_s)

                # trailing vocab chunks
                for te in (nsteps, nsteps + 1):
                    for g in ready_at.get(te, []):
                        load_vchunk(g)
                        emit_vchunk(g, 0, 10)

    nc.compile()
    return nc


def get_n_ts(sizes):
    lens = np.asarray(sizes).astype(np.int64)[:, 0]
    lens_s = lens[np.argsort(-lens, kind="stable")]
    return tuple(int((lens_s > t + 1).sum()) for t in range(T))


def _build_cached(n_ts):
    key = tuple(n_ts)
    if key not in _CACHED:
        _CACHED[key] = build(list(key))
    return _CACHED[key]


def host_prep(feats, sequences, sizes, emb, td_wih, td_whh, td_b,
              lg_wih, lg_whh, lg_b, af_w, af_b, ad_w, ad_b, at_w, at_b,
              out_w, out_b):
    f32 = np.float32
    bf = ml_dtypes.bfloat16
    lens = np.asarray(sizes).astype(np.int64)[:, 0]
    order = np.argsort(-lens, kind="stable")
    seq = np.asarray(sequences).astype(np.int64)[order]
    fts = np.ascontiguousarray(np.asarray(feats, f32)[order])
    favgT_np = np.ascontiguousarray(fts.mean(axis=1).T).astype(bf)

    embs = np.asarray(emb, f32)[seq[:, :T]]
    embsT = np.ascontiguousarray(
        embs.transpose(2, 1, 0)).reshape(E, T * B).astype(bf)

    bdiag = np.tile(np.eye(BSH, dtype=f32), (4, 1)).astype(bf)
    t128 = np.tile(np.eye(BSH, dtype=f32), (4, 4)).astype(bf)

    td_wih = np.asarray(td_wih, f32)
    td_whh = np.asarray(td_whh, f32)
    lg_wih = np.asarray(lg_wih, f32)
    lg_whh = np.asarray(lg_whh, f32)
    af_wT = np.ascontiguousarray(np.asarray(af_w, f32).T).astype(bf)
    afb_full = np.asarray(af_b, f32) + np.asarray(ad_b, f32)
    ad_wv = np.asarray(ad_w, f32)
    adwT_full = np.ascontiguousarray(ad_wv.T).astype(bf)
    eye32_np = np.eye(BSH, dtype=f32).astype(bf)

    def bsel_k(k):
        m = np.zeros((B, BSH), f32)
        m[np.arange(k * BSH, (k + 1) * BSH), np.arange(BSH)] = 1.0
        return m.astype(bf)
    atwT = np.ascontiguousarray(np.asarray(at_w, f32).T).astype(bf)
    out_wv = np.asarray(out_w, f32)
    out_bv = np.asarray(out_b, f32)

    in_maps = []
    for k in range(NC):
        gsl = np.concatenate([np.arange(g * D + k * 128, g * D + (k + 1) * 128)
                              for g in range(4)])
        bsl = slice(k * BSH, (k + 1) * BSH)
        fsh = fts[bsl]
        ftsT_k = np.ascontiguousarray(
            fsh.transpose(2, 1, 0).reshape(F, RB)).astype(bf)
        ftsrb_k = np.ascontiguousarray(
            fsh.transpose(1, 0, 2).reshape(RB, F)).astype(bf)
        ow_pad = np.zeros((VSH, D), f32)
        ow_pad[:1250] = out_wv[k * 1250:(k + 1) * 1250]
        ob_pad = np.zeros((VSH, 1), f32)
        ob_pad[:1250, 0] = out_bv[k * 1250:(k + 1) * 1250]
        in_maps.append({
            "wtdh2": np.ascontiguousarray(td_wih[gsl, 0:D].T).astype(bf),
            "wtdh1": np.ascontiguousarray(td_whh[gsl].T).astype(bf),
            "wfavg": np.ascontiguousarray(td_wih[gsl, D:D + F].T).astype(bf),
            "we": np.ascontiguousarray(td_wih[gsl, D + F:].T).astype(bf),
            "tdb": np.ascontiguousarray(np.asarray(td_b, f32)[gsl][None, :]),
            "wlgaw": np.ascontiguousarray(lg_wih[gsl, 0:F].T).astype(bf),
            "wlgh1": np.ascontiguousarray(lg_wih[gsl, F:].T).astype(bf),
            "wlgh2": np.ascontiguousarray(lg_whh[gsl].T).astype(bf),
            "lgb": np.ascontiguousarray(np.asarray(lg_b, f32)[gsl][None, :]),
            "adwT": adwT_full,
            "bsel": bsel_k(k),
            "eye32": eye32_np,
            "afw": af_wT,
            "afb": np.ascontiguousarray(afb_full[None, :]).astype(bf),
            "atw": atwT,
            "ftsT": ftsT_k,
            "favgT": favgT_np,
            "ftsrb": ftsrb_k,
            "embsT": embsT,
            "outw": np.ascontiguousarray(ow_pad.T).astype(bf),
            "outb": ob_pad,
            "bdiag": bdiag,
            "t128": t128,
        })
    return in_maps


def postprocess(res, n_ts):
    shards = [res.results[k]["outp"].reshape(VSH, T, B)[:1250]
              for k in range(NC)]
    full = np.concatenate(shards, axis=0)
    for t in range(T):
        full[:, t, n_ts[t]:] = 0.0
    return np.ascontiguousarray(full.transpose(2, 1, 0))


def kernel(**inputs):
    n_ts = get_n_ts(inputs["sizes"])
    in_maps = host_prep(**inputs)
    nc = _build_cached(n_ts)
    res = run_bass_kernel_spmd(nc, in_maps, core_ids=list(range(NC)))
    return postprocess(res, n_ts)

